# revision 39
# baseline (speedup 1.0000x reference)
"""Multi-head self-attention (RoPE, causal) on 8 trn2 NeuronCores.

Sharding: core c -> batch b = c // 4, head group g = c % 4 (4 heads each).
Each core:
  - projects Q,K,V for its batch / its 4 heads (token-major; x and the
    weights arrive split across both HWDGE queues), applies RoPE via a
    gathered cos/sin table, transposes Q,K to [dk, S] layout one chunk
    behind the matmul stream (PSUM->SBUF copies ride the scalar engine),
  - computes scores^T = K^T-chunk x Q (k on partitions) with the two heads
    of a pair interleaved so consecutive matmuls land on PE row strips
    0/64, exp on ACT into persistent per-chunk tiles,
  - accumulates out^T[d, q] in PSUM across the whole causal k-range per
    512-token q-chunk (V' weights padded to 128 columns for fast weight
    load, softmax sums ride along as row 64 via an appended ones-column);
    the sums row is reciprocated per-chunk right after each accumulation
    (partition-0 scratch: custom DVE ops miswrite at nonzero base
    partitions on HW); each chunk's normalize (ones-broadcast matmul +
    one multiply into a small scratch tile) trails its AV by one unit so
    the PE never stalls on the DVE reciprocal chain, and the normalized
    chunk is immediately staged to DRAM,
  - exchanges the head-sharded attention output via one 8-core AllToAll
    per head pair (blocks duplicated for both batch halves so the SPMD
    program stays uniform; a tiny early dummy collective absorbs launch
    skew / CC warmup).  A per-core [128, 4] row-index table drives
    contiguous-block indirect gathers of the peers' rows,
  - output projection accumulates all 8 d-chunks into a persistent
    8-bank PSUM accumulator (attention PSUM pools are closed first),
    dp-major with pair-0 chunks first so they overlap the pair-1
    collective, then streams the bf16 result out on two queues.
Host only reshapes/transposes/casts inputs and concatenates outputs.
"""

import contextlib
import os
import sys

for _p in ("/opt/trn_rl_repo",):
    if _p not in sys.path:
        sys.path.append(_p)

_NOILV = bool(os.environ.get("K_NOILV"))      # bisect: no head interleave
_NOPAD = bool(os.environ.get("K_NOPAD"))      # bisect: 65-col AV weights
_DBG = bool(os.environ.get("K_DBG"))          # dump outv/attT for pair 0
_KW = int(os.environ.get("K_KW", "0"))        # PE keep-warm matmul count
_NODUM = bool(os.environ.get("K_NODUM"))      # bisect: skip dummy collective

import numpy as np
import ml_dtypes

import concourse.bass as bass
import concourse.mybir as mybir
import concourse.tile as tile
from concourse import bacc
from concourse.bass import ds, ts
from concourse.bass_utils import run_bass_kernel_spmd
from concourse.masks import make_identity

BF16 = mybir.dt.bfloat16
F32 = mybir.dt.float32
I32 = mybir.dt.int32

B, S, D = 2, 2048, 1024
H, DK = 16, 64
THETA = 10000.0
MAXPOS = 2048
N_CORES = 8
GROUPS = 4          # head groups (cores) per batch
HPC = H // GROUPS   # heads per core = 4
QKV_COLS = 3 * HPC * DK        # 768 per-core projection width
QK_COLS = 2 * HPC * DK         # 512 (Q then K)
NSC = S // 128                 # 16 token chunks
NQC = S // 512                 # 4 q column-chunks
QSLICE = S // GROUPS           # 512 output tokens per core
VW = DK + 1                    # V cols + ones col per head
VROW = HPC * VW + 128 - VW     # 323: head h weight = cols [65h, 65h+128)
MUL = mybir.AluOpType.mult
ADD = mybir.AluOpType.add
SUB = mybir.AluOpType.subtract


def _build():
    nc = bacc.Bacc("TRN2", num_devices=N_CORES)

    # inputs host-prearranged to partition-major [128, k*cols] so each DMA
    # is 128 large contiguous descriptors instead of 512-1024 small ones
    xT = nc.dram_tensor("xT", [128, 8 * S], BF16, kind="ExternalInput")
    wqkvT = nc.dram_tensor("wqkvT", [128, 8 * QKV_COLS], BF16,
                           kind="ExternalInput")
    woT = nc.dram_tensor("woT", [128, 8 * D], BF16, kind="ExternalInput")
    cstab = nc.dram_tensor("cstab", [MAXPOS, 2 * DK], F32, kind="ExternalInput")
    pos = nc.dram_tensor("pos", [128, NSC], I32, kind="ExternalInput")
    tri = nc.dram_tensor("tri", [128, 128], BF16, kind="ExternalInput")
    ridx = nc.dram_tensor("ridx", [128, GROUPS], I32, kind="ExternalInput")
    finT = nc.dram_tensor("finT", [D, QSLICE], BF16, kind="ExternalOutput")
    if _DBG:
        dbgov = nc.dram_tensor("dbgov", [DK + 1, 2 * S], F32,
                               kind="ExternalOutput")
        dbgraw = nc.dram_tensor("dbgraw", [DK + 1, 2 * S], F32,
                                kind="ExternalOutput")
        dbget = nc.dram_tensor("dbget", [128, S], BF16,
                               kind="ExternalOutput")
        dbgat = nc.dram_tensor("dbgat", [128, 2 * S], BF16,
                               kind="ExternalOutput")

    with tile.TileContext(nc) as tc:
        with (
            tc.tile_pool(name="const", bufs=1) as constp,
            tc.tile_pool(name="wts", bufs=1) as wtsp,
            tc.tile_pool(name="seq", bufs=1) as seqp,
            tc.tile_pool(name="xtp", bufs=3) as xtp,
            tc.tile_pool(name="ropet", bufs=2) as ropet,
            tc.tile_pool(name="attp", bufs=1) as attp,
            tc.tile_pool(name="stg", bufs=4) as stgp,
            tc.tile_pool(name="dram", bufs=1, space="DRAM") as dramp,
        ):
            # PSUM pools for projection + attention; closed (via the stack)
            # before the output projection so its 8-bank accumulator fits
            pstack = contextlib.ExitStack()
            pbig = pstack.enter_context(
                tc.tile_pool(name="pbig", bufs=3, space="PSUM"))
            psmall = pstack.enter_context(
                tc.tile_pool(name="psmall", bufs=2, space="PSUM"))
            # ---------------- constants + resident weights ----------------
            ident = constp.tile([128, 128], BF16)
            make_identity(nc, ident[:])
            tri_t = constp.tile([128, 128], BF16)
            nc.sync.dma_start(out=tri_t[:], in_=tri[:])
            ones1 = constp.tile([1, 64], BF16)
            nc.vector.memset(ones1[:], 1.0)

            # weights first, split across both queues, so the first
            # projection matmul's inputs land as early as possible
            wt = wtsp.tile([128, 8, QKV_COLS], BF16)       # [dchunk][768]
            nc.sync.dma_start(
                out=wt[0:64, :, :],
                in_=wqkvT[0:64, :].rearrange("p (k e) -> p k e", k=8))
            nc.scalar.dma_start(
                out=wt[64:128, :, :],
                in_=wqkvT[64:128, :].rearrange("p (k e) -> p k e", k=8))
            wo = wtsp.tile([128, 8, D], BF16)   # loaded after projection

            if not _NODUM:
                # tiny dummy collective fired at t~=0: absorbs the inter-core
                # launch skew + CC warmup so the real collectives' entry
                # barriers are short when they matter (tail critical path)
                dum_in = dramp.tile([N_CORES, 64], BF16, name="dumin")
                dum_out = dramp.tile([N_CORES, 64], BF16, name="dumout")
                nc.gpsimd.collective_compute(
                    "AllToAll",
                    mybir.AluOpType.bypass,
                    ins=[dum_in[:]],
                    outs=[dum_out[:]],
                    replica_groups=[list(range(N_CORES))],
                )

            # persistent per-core tensors
            qt = seqp.tile([128, 2, S], BF16)   # Q^T  [pair, dk(2x64), q]
            kt = seqp.tile([128, 2, S], BF16)   # K^T
            vv = seqp.tile([128, NSC, VROW], BF16)   # V + ones col + pad
            vv4 = vv[:, :, 0:HPC * VW].rearrange("p c (h e) -> p c h e", h=HPC)
            nc.vector.memset(vv4[:, :, :, DK:DK + 1], 1.0)
            nc.vector.memset(vv[:, :, HPC * VW:VROW], 0.0)
            outv = attp.tile([DK + 1, 2, S], F32)  # vals + sums (pair slot)
            rwo = attp.tile([128, 8, QSLICE], BF16)
            ridx_sb = attp.tile([128, GROUPS], I32)
            srw = attp.tile([1, 2, S], BF16)    # per-pair recip rows
            sct = attp.tile([1, 512], F32)      # partition-0 recip scratch

            # ------------- projection + RoPE + transposes (scoped x) -------
            with tc.tile_pool(name="xap", bufs=1) as xap:
                xa = xap.tile([128, 8, S], BF16)           # resident x^T
                # partition-split across the two queues: 64 contiguous
                # 32KB descriptors each
                nc.scalar.dma_start(
                    out=xa[0:64, :, :],
                    in_=xT[0:64, :].rearrange("p (k s) -> p k s", k=8))
                nc.sync.dma_start(
                    out=xa[64:128, :, :],
                    in_=xT[64:128, :].rearrange("p (k s) -> p k s", k=8))
                cs = xap.tile([128, NSC, 2 * DK], F32)     # cos/sin gather
                pidx = xap.tile([128, NSC], I32)
                nc.sync.dma_start(out=pidx[:], in_=pos[:])
                for c in range(NSC):
                    nc.gpsimd.indirect_dma_start(
                        out=cs[:, c, :],
                        out_offset=None,
                        in_=cstab[:],
                        in_offset=bass.IndirectOffsetOnAxis(
                            ap=pidx[:, c:c + 1], axis=0),
                    )

                def emit_transposes(sc, roped):
                    for t in range(4):
                        tp = psmall.tile([128, 128], BF16, space="PSUM",
                                         tag="small")
                        nc.tensor.transpose(tp[:], roped[:, ts(t, 128)],
                                            ident[:])
                        dst = qt if t < 2 else kt
                        nc.scalar.copy(dst[:, t % 2, ts(sc, 128)], tp[:])

                pend_tp = None
                for sc in range(NSC):
                    ps = pbig.tile([128, QKV_COLS], F32, space="PSUM",
                                   tag="big")
                    for k in range(8):
                        nc.tensor.matmul(
                            ps[:, 0:512], lhsT=xa[:, k, ts(sc, 128)],
                            rhs=wt[:, k, 0:512],
                            start=(k == 0), stop=(k == 7),
                        )
                    for k in range(8):
                        nc.tensor.matmul(
                            ps[:, 512:768], lhsT=xa[:, k, ts(sc, 128)],
                            rhs=wt[:, k, 512:768],
                            start=(k == 0), stop=(k == 7),
                        )
                    if pend_tp is not None:
                        emit_transposes(*pend_tp)

                    # RoPE over the Q,K halves (cols 0:512), 8 blocks of 64
                    def qk_ap(off):
                        a = ps[:]
                        return bass.AP(a.tensor, a.offset + off,
                                       [a.ap[0], [DK, 2 * HPC], [2, DK // 2]])

                    def cs_ap(off):
                        a = cs[:, sc, :]
                        return bass.AP(a.tensor, a.offset + off,
                                       [a.ap[0], [0, 2 * HPC], [2, DK // 2]])

                    t1 = ropet.tile([128, 2 * HPC, DK // 2], F32, tag="t1")
                    t2 = ropet.tile([128, 2 * HPC, DK // 2], F32, tag="t2")
                    t3 = ropet.tile([128, 2 * HPC, DK // 2], F32, tag="t3")
                    t4 = ropet.tile([128, 2 * HPC, DK // 2], F32, tag="t4")
                    roped = ropet.tile([128, QK_COLS], BF16, tag="roped")

                    def roped_ap(off):
                        a = roped[:]
                        return bass.AP(a.tensor, a.offset + off,
                                       [a.ap[0], [DK, 2 * HPC], [2, DK // 2]])

                    nc.vector.tensor_tensor(t1[:], qk_ap(0), cs_ap(0), MUL)
                    nc.vector.tensor_tensor(t2[:], qk_ap(1), cs_ap(DK), MUL)
                    nc.vector.tensor_tensor(roped_ap(0), t1[:], t2[:], SUB)
                    nc.vector.tensor_tensor(t3[:], qk_ap(0), cs_ap(DK), MUL)
                    nc.vector.tensor_tensor(t4[:], qk_ap(1), cs_ap(0), MUL)
                    nc.vector.tensor_tensor(roped_ap(1), t3[:], t4[:], ADD)

                    # V columns (ones col + pad already set); ACT copy keeps
                    # the vector engine free for the RoPE chain
                    nc.scalar.copy(
                        vv4[:, sc, :, 0:DK],
                        ps[:, 512:768].rearrange("p (h e) -> p h e", h=HPC),
                    )

                    # transpose roped q/k after the NEXT chunk's matmuls so
                    # the PE never waits on the vector engine
                    pend_tp = (sc, roped)
                emit_transposes(*pend_tp)

                # wo load deferred here: overlaps attention, clear of the
                # startup critical path
                nc.scalar.dma_start(
                    out=wo[:, :, :],
                    in_=woT[:, :].rearrange("p (k e) -> p k e", k=8))

            # ------------- attention (head pairs) + split AllToAll ---------
            # a2ain[p] rows (u*512 + j*128 + d): q-slice j's rows for this
            # core's pair p, duplicated u=0/1 so block (4b + j) is correct
            # for either batch half b (SPMD-uniform staging).  After the
            # 8-core AllToAll, out block (4b + g) holds core (b, g)'s pair-p
            # rows for THIS core's q-slice == d-model chunk dp = 2g + p.
            a2ain = [dramp.tile([N_CORES * 128, QSLICE], BF16,
                                name=f"a2ain{p}") for p in range(2)]
            a2aout = [dramp.tile([N_CORES * 128, QSLICE], BF16,
                                 name=f"a2aout{p}") for p in range(2)]

            with tc.tile_pool(name="post", bufs=1) as postp:
                def emit_scores(p, j, eta, etb):
                    """Score matmuls + exp for chunk j, both heads of pair p
                    interleaved (row strips 0 / 64)."""
                    for qh in range(2):
                        q0 = max(128 * j, 1024 * qh)
                        q1 = 1024 * (qh + 1)
                        if q0 >= q1:
                            continue
                        sps = []
                        for i in range(2):
                            sp = pbig.tile([128, 1024], F32, space="PSUM",
                                           tag="big", name=f"sp{i}")
                            sps.append(sp)
                        if _NOILV:
                            for i, hf in enumerate((0, DK)):
                                for qq in (1024 * qh, 1024 * qh + 512):
                                    a, bnd = max(q0, qq), min(q1, qq + 512)
                                    if a >= bnd:
                                        continue
                                    nc.tensor.matmul(
                                        sps[i][:, ds(a - 1024 * qh, bnd - a)],
                                        lhsT=kt[ds(hf, DK), p, ts(j, 128)],
                                        rhs=qt[ds(hf, DK), p, ds(a, bnd - a)],
                                        start=True, stop=True,
                                    )
                        else:
                            for qq in (1024 * qh, 1024 * qh + 512):
                                a, bnd = max(q0, qq), min(q1, qq + 512)
                                if a >= bnd:
                                    continue
                                for i, hf in enumerate((0, DK)):
                                    nc.tensor.matmul(
                                        sps[i][:, ds(a - 1024 * qh, bnd - a)],
                                        lhsT=kt[ds(hf, DK), p, ts(j, 128)],
                                        rhs=qt[ds(hf, DK), p, ds(a, bnd - a)],
                                        start=True, stop=True,
                                    )
                        for i, et in enumerate((eta, etb)):
                            nc.scalar.activation(
                                et[:, ds(q0 - 128 * j, q1 - q0)],
                                sps[i][:, ds(q0 - 1024 * qh, q1 - q0)],
                                mybir.ActivationFunctionType.Exp,
                            )
                    # mask the diagonal block (q < k -> 0)
                    for et in (eta, etb):
                        nc.vector.tensor_tensor(
                            et[:, 0:128], et[:, 0:128], tri_t[:], MUL)

                def emit_av(h, qc, ets):
                    """Accumulate out^T for q-chunk qc over the whole causal
                    k-range in one PSUM tile."""
                    jmax = 4 * qc + 3
                    lw = VW if _NOPAD else 128
                    part = psmall.tile([128, 512], F32, space="PSUM",
                                       tag="small")
                    for j in range(jmax + 1):
                        a = max(512 * qc, 128 * j)
                        w = 512 * (qc + 1) - a
                        nc.tensor.matmul(
                            part[0:lw, ds(a - 512 * qc, w)],
                            lhsT=vv[:, j, ds(VW * h, lw)],
                            rhs=ets[j][:, ds(a - 128 * j, w)],
                            start=(j == 0), stop=(j == jmax),
                        )
                    nc.vector.tensor_copy(
                        outv[0:DK, h % 2, ts(qc, 512)], part[0:DK, :])
                    # pipelined per-chunk reciprocal of the sums row; the
                    # custom DVE op only works at partition 0 / offset 0
                    nc.vector.tensor_copy(sct[:], part[ds(DK, 1), :])
                    nc.vector.reciprocal_approx_fast(sct[:], sct[:])
                    nc.vector.tensor_copy(srw[:, h % 2, ts(qc, 512)], sct[:])

                def emit_norm_stage(p, h, qc):
                    """Normalize one q-chunk of head h into a scratch tile
                    and immediately stage it into the AllToAll input
                    (duplicated u=0/1), so the pair's collective can trigger
                    the moment its last AV chunk lands.  Scratch tiles (not
                    one big attT) keep the stage-DMA reads from serializing
                    against later chunks' normalize writes."""
                    hf = (h % 2) * DK
                    rb = psmall.tile([DK, 512], F32, space="PSUM",
                                     tag="small")
                    nc.tensor.matmul(rb[:], lhsT=ones1[:],
                                     rhs=srw[:, h % 2, ts(qc, 512)],
                                     start=True, stop=True)
                    sg = stgp.tile([DK, 512], BF16, tag="sg")
                    nc.vector.tensor_tensor(
                        sg[:], outv[0:DK, h % 2, ts(qc, 512)], rb[:], MUL)
                    stage = a2ain[p][:].rearrange(
                        "(u j d) q -> u d j q", u=2, j=GROUPS)
                    for u in range(2):
                        nc.sync.dma_start(
                            out=stage[u, ds(hf, DK), qc, :],
                            in_=sg[:],
                        )

                # gather-offset table [r, g] = 128*(4b + g) + r; single
                # small DMA on the gpsimd queue (idle through attention)
                nc.gpsimd.dma_start(out=ridx_sb[:, :], in_=ridx[:, :])

                with tc.tile_pool(name="etp", bufs=1) as etp:
                    for p in range(2):
                        ha, hb = 2 * p, 2 * p + 1
                        eta, etb = [], []
                        for j in range(NSC):
                            w = S - 128 * j
                            eta.append(etp.tile([128, w], BF16,
                                                name=f"eta{j}",
                                                tag=f"eta{j}"))
                            etb.append(etp.tile([128, w], BF16,
                                                name=f"etb{j}",
                                                tag=f"etb{j}"))
                        # norm+stage trails its AV by one unit so the PE
                        # never waits on the DVE reciprocal chain between
                        # consecutive AV accumulations (keeps PE warm, and
                        # the pair's collective triggers ~1 unit after the
                        # last AV instead of serializing 4 units)
                        avsched = {7: (ha, 0), 8: (hb, 0), 11: (ha, 1),
                                   12: (hb, 1), 13: (ha, 2), 14: (hb, 2)}
                        pend_ns = None
                        for j in range(NSC):
                            emit_scores(p, j, eta[j], etb[j])
                            if j in avsched:
                                h, qc = avsched[j]
                                emit_av(h, qc, eta if h == ha else etb)
                                if pend_ns is not None:
                                    emit_norm_stage(p, *pend_ns)
                                pend_ns = (h, qc)
                        emit_av(ha, 3, eta)
                        emit_norm_stage(p, *pend_ns)
                        emit_av(hb, 3, etb)
                        emit_norm_stage(p, ha, 3)
                        emit_norm_stage(p, hb, 3)
                        if _DBG and p == 0:
                            nc.sync.dma_start(
                                out=dbgraw[:].rearrange(
                                    "p (a b) -> p a b", a=2),
                                in_=outv[:, :, :])
                            nc.sync.dma_start(out=dbget[:], in_=eta[0][:])
                        nc.gpsimd.collective_compute(
                            "AllToAll",
                            mybir.AluOpType.bypass,
                            ins=[a2ain[p][:]],
                            outs=[a2aout[p][:]],
                            replica_groups=[list(range(N_CORES))],
                        )

                # ------------- output projection (q-slice) ------------------
                # Release the attention PSUM pools and claim all 8 banks as
                # one persistent accumulator: each ec-chunk accumulates its
                # full 8-chunk contraction in its own bank, dp-major so the
                # even-dp half (pair 0, available early) runs during the
                # pair-1 collective wait, and each odd-dp gather overlaps the
                # previous dp's matmuls.  The bank reuse also hard-orders
                # this work after attention, keeping the scheduler from
                # stealing PE time from the collective's critical path.
                pstack.close()
                with tc.tile_pool(name="ptail", bufs=8, space="PSUM") as pt:
                    fps = [pt.tile([128, QSLICE], F32, space="PSUM",
                                   tag="fp", name=f"fp{ec}")
                           for ec in range(8)]
                    dporder = (0, 2, 4, 6, 1, 3, 5, 7)
                    for i, dp in enumerate(dporder):
                        nc.gpsimd.indirect_dma_start(
                            out=rwo[:, dp, :],
                            out_offset=None,
                            in_=a2aout[dp % 2][:],
                            in_offset=bass.IndirectOffsetOnAxis(
                                ap=ridx_sb[:, dp // 2:dp // 2 + 1], axis=0),
                        )
                        for ec in range(8):
                            nc.tensor.matmul(
                                fps[ec][:], lhsT=wo[:, dp, ts(ec, 128)],
                                rhs=rwo[:, dp, :],
                                start=(i == 0), stop=(i == 7),
                            )
                    for ec in range(8):
                        fin_sb = xtp.tile([128, QSLICE], BF16, tag="fin")
                        # alternate engines/queues so the 8 stores pipeline
                        if ec % 2 == 0:
                            nc.vector.tensor_copy(fin_sb[:], fps[ec][:])
                            nc.sync.dma_start(out=finT[ts(ec, 128), :],
                                              in_=fin_sb[:])
                        else:
                            nc.scalar.copy(fin_sb[:], fps[ec][:])
                            nc.scalar.dma_start(out=finT[ts(ec, 128), :],
                                                in_=fin_sb[:])

    nc.compile()
    return nc


def _pmaj(a):
    """[D, C] -> partition-major [128, 8*C]: row 128k+p -> [p, k*C:...]."""
    d, c = a.shape
    return np.ascontiguousarray(
        a.reshape(8, 128, c).transpose(1, 0, 2).reshape(128, 8 * c))


def _host_prep(x, token_positions, W_qkv, W_o):
    bf16 = ml_dtypes.bfloat16
    xT = np.transpose(x, (0, 2, 1)).astype(bf16)                      # [B,D,S]
    xTr = [_pmaj(xT[b]) for b in range(B)]

    # per-group W_qkv^T slices (Q rows pre-scaled by 1/sqrt(dk))
    wq = W_qkv[0 * D:1 * D] * np.float32(1.0 / np.sqrt(DK))
    wk = W_qkv[1 * D:2 * D]
    wv = W_qkv[2 * D:3 * D]
    wslices = []
    for g in range(GROUPS):
        rows = slice(g * HPC * DK, (g + 1) * HPC * DK)
        wsl = np.concatenate([wq[rows], wk[rows], wv[rows]], axis=0)  # [768, D]
        wslices.append(_pmaj(wsl.T.astype(bf16)))                     # [128,6144]

    woT = _pmaj(W_o.T.astype(bf16))                                   # [128,8192]

    idx = np.arange(DK // 2, dtype=np.float64)
    freqs = 1.0 / (THETA ** (2.0 * idx / DK))
    ang = np.arange(MAXPOS, dtype=np.float64)[:, None] * freqs[None, :]
    cstab = np.zeros((MAXPOS, 2 * DK), dtype=np.float32)
    cstab[:, 0:DK:2] = np.cos(ang)
    cstab[:, 1:DK:2] = np.cos(ang)
    cstab[:, DK::2] = np.sin(ang)
    cstab[:, DK + 1::2] = np.sin(ang)

    tri = (np.arange(128)[None, :] >= np.arange(128)[:, None]).astype(bf16)

    # pos pre-rearranged to [partition, chunk]: pos_r[p, c] = pos[128c + p]
    # (contiguous 64B per partition -> cheap DMA)
    posi = np.ascontiguousarray(
        np.asarray(token_positions).astype(np.int32).reshape(B, NSC, 128)
        .transpose(0, 2, 1))

    # ridx[r, g'] = row 128*(4b + g') + r of a2aout[p]: source core (b, g')'s
    # pair-p rows for this core's q-slice = d-model chunk dp = 2g' + p.
    rr = np.arange(128)
    in_maps = []
    for c in range(N_CORES):
        b, g = c // GROUPS, c % GROUPS
        ridx = 128 * (4 * b + np.arange(GROUPS))[None, :] + rr[:, None]
        in_maps.append({
            "xT": xTr[b],
            "wqkvT": wslices[g],
            "woT": woT,
            "cstab": cstab,
            "pos": np.ascontiguousarray(posi[b]),
            "tri": tri,
            "ridx": np.ascontiguousarray(ridx.astype(np.int32)),
        })
    return in_maps


def _assemble(results):
    out = np.empty((B, S, D), dtype=np.float32)
    for b in range(B):
        fullT = np.concatenate(
            [results[b * GROUPS + g]["finT"].astype(np.float32)
             for g in range(GROUPS)], axis=1)
        out[b] = fullT.T
    return out


_NC_CACHE = {}


def run(inputs, trace=False, **kw):
    if "nc" not in _NC_CACHE:
        _NC_CACHE["nc"] = _build()
    nc = _NC_CACHE["nc"]
    in_maps = _host_prep(**inputs)
    res = run_bass_kernel_spmd(
        nc, in_maps, core_ids=list(range(N_CORES)), trace=trace, **kw)
    return _assemble(res.results), res


def kernel(**inputs):
    out, _ = run(inputs, trace=False)
    return out



# revision 40
# speedup vs baseline: 1.0369x; 1.0369x over previous
"""Multi-head self-attention (RoPE, causal) on 8 trn2 NeuronCores.

Sharding: core c -> batch b = c // 4, head group g = c % 4 (4 heads each).
Each core:
  - projects Q,K,V for its batch / its 4 heads (token-major; x and the
    weights arrive split across both HWDGE queues), applies RoPE via a
    gathered cos/sin table, transposes Q,K to [dk, S] layout one chunk
    behind the matmul stream (PSUM->SBUF copies ride the scalar engine),
  - computes scores^T = K^T-chunk x Q (k on partitions) with the two heads
    of a pair interleaved so consecutive matmuls land on PE row strips
    0/64, exp on ACT into persistent per-chunk tiles,
  - accumulates out^T[d, q] in PSUM across the whole causal k-range per
    512-token q-chunk (V' weights padded to 128 columns for fast weight
    load, softmax sums ride along as row 64 via an appended ones-column);
    the sums row is reciprocated per-chunk right after each accumulation
    (partition-0 scratch: custom DVE ops miswrite at nonzero base
    partitions on HW); each chunk's normalize (ones-broadcast matmul +
    one multiply into a small scratch tile) trails its AV by one unit so
    the PE never stalls on the DVE reciprocal chain, and the normalized
    chunk is immediately staged to DRAM,
  - exchanges the head-sharded attention output via one 8-core AllToAll
    per head pair (blocks duplicated for both batch halves so the SPMD
    program stays uniform; a tiny early dummy collective absorbs launch
    skew / CC warmup).  A per-core [128, 4] row-index table drives
    contiguous-block indirect gathers of the peers' rows,
  - output projection accumulates all 8 d-chunks into a persistent
    8-bank PSUM accumulator (attention PSUM pools are closed first),
    dp-major with pair-0 chunks first so they overlap the pair-1
    collective, then streams the bf16 result out on two queues.
Host only reshapes/transposes/casts inputs and concatenates outputs.
"""

import contextlib
import os
import sys

for _p in ("/opt/trn_rl_repo",):
    if _p not in sys.path:
        sys.path.append(_p)

_NOILV = bool(os.environ.get("K_NOILV"))      # bisect: no head interleave
_NOPAD = bool(os.environ.get("K_NOPAD"))      # bisect: 65-col AV weights
_DBG = bool(os.environ.get("K_DBG"))          # dump outv/attT for pair 0
_KW = int(os.environ.get("K_KW", "0"))        # PE keep-warm matmul count
_NODUM = bool(os.environ.get("K_NODUM"))      # bisect: skip dummy collective

import numpy as np
import ml_dtypes

import concourse.bass as bass
import concourse.mybir as mybir
import concourse.tile as tile
from concourse import bacc
from concourse.bass import ds, ts
from concourse.bass_utils import run_bass_kernel_spmd
from concourse.masks import make_identity

BF16 = mybir.dt.bfloat16
F32 = mybir.dt.float32
I32 = mybir.dt.int32

B, S, D = 2, 2048, 1024
H, DK = 16, 64
THETA = 10000.0
MAXPOS = 2048
N_CORES = 8
GROUPS = 4          # head groups (cores) per batch
HPC = H // GROUPS   # heads per core = 4
QKV_COLS = 3 * HPC * DK        # 768 per-core projection width
QK_COLS = 2 * HPC * DK         # 512 (Q then K)
NSC = S // 128                 # 16 token chunks
NQC = S // 512                 # 4 q column-chunks
QSLICE = S // GROUPS           # 512 output tokens per core
VW = DK + 1                    # V cols + ones col per head
VROW = HPC * VW + 128 - VW     # 323: head h weight = cols [65h, 65h+128)
MUL = mybir.AluOpType.mult
ADD = mybir.AluOpType.add
SUB = mybir.AluOpType.subtract


def _build():
    nc = bacc.Bacc("TRN2", num_devices=N_CORES)

    xT = nc.dram_tensor("xT", [D, S], BF16, kind="ExternalInput")
    wqkvT = nc.dram_tensor("wqkvT", [D, QKV_COLS], BF16, kind="ExternalInput")
    woT = nc.dram_tensor("woT", [D, D], BF16, kind="ExternalInput")
    cstab = nc.dram_tensor("cstab", [MAXPOS, 2 * DK], F32, kind="ExternalInput")
    pos = nc.dram_tensor("pos", [128, NSC], I32, kind="ExternalInput")
    tri = nc.dram_tensor("tri", [128, 128], BF16, kind="ExternalInput")
    ridx = nc.dram_tensor("ridx", [128, GROUPS], I32, kind="ExternalInput")
    finT = nc.dram_tensor("finT", [D, QSLICE], BF16, kind="ExternalOutput")
    if _DBG:
        dbgov = nc.dram_tensor("dbgov", [DK + 1, 2 * S], F32,
                               kind="ExternalOutput")
        dbgraw = nc.dram_tensor("dbgraw", [DK + 1, 2 * S], F32,
                                kind="ExternalOutput")
        dbget = nc.dram_tensor("dbget", [128, S], BF16,
                               kind="ExternalOutput")
        dbgat = nc.dram_tensor("dbgat", [128, 2 * S], BF16,
                               kind="ExternalOutput")

    with tile.TileContext(nc) as tc:
        with (
            tc.tile_pool(name="const", bufs=1) as constp,
            tc.tile_pool(name="wts", bufs=1) as wtsp,
            tc.tile_pool(name="seq", bufs=1) as seqp,
            tc.tile_pool(name="xtp", bufs=3) as xtp,
            tc.tile_pool(name="ropet", bufs=2) as ropet,
            tc.tile_pool(name="attp", bufs=1) as attp,
            tc.tile_pool(name="stg", bufs=4) as stgp,
            tc.tile_pool(name="dram", bufs=1, space="DRAM") as dramp,
        ):
            # PSUM pools for projection + attention; closed (via the stack)
            # before the output projection so its 8-bank accumulator fits
            pstack = contextlib.ExitStack()
            pbig = pstack.enter_context(
                tc.tile_pool(name="pbig", bufs=3, space="PSUM"))
            psmall = pstack.enter_context(
                tc.tile_pool(name="psmall", bufs=2, space="PSUM"))
            # ---------------- constants + resident weights ----------------
            ident = constp.tile([128, 128], BF16)
            make_identity(nc, ident[:])
            tri_t = constp.tile([128, 128], BF16)
            nc.sync.dma_start(out=tri_t[:], in_=tri[:])
            ones1 = constp.tile([1, 64], BF16)
            nc.vector.memset(ones1[:], 1.0)

            # weights first, split across both queues, so the first
            # projection matmul's inputs land as early as possible
            wt = wtsp.tile([128, 8, QKV_COLS], BF16)       # [dchunk][768]
            nc.sync.dma_start(
                out=wt[:, 0:4, :],
                in_=wqkvT[0:512].rearrange("(k p) e -> p k e", p=128))
            nc.scalar.dma_start(
                out=wt[:, 4:8, :],
                in_=wqkvT[512:1024].rearrange("(k p) e -> p k e", p=128))
            wo = wtsp.tile([128, 8, D], BF16)   # loaded after projection

            if not _NODUM:
                # tiny dummy collective fired at t~=0: absorbs the inter-core
                # launch skew + CC warmup so the real collectives' entry
                # barriers are short when they matter (tail critical path)
                dum_in = dramp.tile([N_CORES, 64], BF16, name="dumin")
                dum_out = dramp.tile([N_CORES, 64], BF16, name="dumout")
                nc.gpsimd.collective_compute(
                    "AllToAll",
                    mybir.AluOpType.bypass,
                    ins=[dum_in[:]],
                    outs=[dum_out[:]],
                    replica_groups=[list(range(N_CORES))],
                )

            # persistent per-core tensors
            qt = seqp.tile([128, 2, S], BF16)   # Q^T  [pair, dk(2x64), q]
            kt = seqp.tile([128, 2, S], BF16)   # K^T
            vv = seqp.tile([128, NSC, VROW], BF16)   # V + ones col + pad
            vv4 = vv[:, :, 0:HPC * VW].rearrange("p c (h e) -> p c h e", h=HPC)
            nc.vector.memset(vv4[:, :, :, DK:DK + 1], 1.0)
            nc.vector.memset(vv[:, :, HPC * VW:VROW], 0.0)
            outv = attp.tile([DK + 1, 2, S], F32)  # vals + sums (pair slot)
            rwo = attp.tile([128, 8, QSLICE], BF16)
            ridx_sb = attp.tile([128, GROUPS], I32)
            srw = attp.tile([1, 2, S], BF16)    # per-pair recip rows
            sct = attp.tile([1, 512], F32)      # partition-0 recip scratch

            # ------------- projection + RoPE + transposes (scoped x) -------
            with tc.tile_pool(name="xap", bufs=1) as xap:
                xa = xap.tile([128, 8, S], BF16)           # resident x^T
                # one DMA per queue half: avoids per-kick queue latency
                nc.scalar.dma_start(
                    out=xa[:, 0:4, :],
                    in_=xT[0:512].rearrange("(k p) s -> p k s", p=128))
                nc.sync.dma_start(
                    out=xa[:, 4:8, :],
                    in_=xT[512:1024].rearrange("(k p) s -> p k s", p=128))
                cs = xap.tile([128, NSC, 2 * DK], F32)     # cos/sin gather
                pidx = xap.tile([128, NSC], I32)
                nc.sync.dma_start(out=pidx[:], in_=pos[:])
                for c in range(NSC):
                    nc.gpsimd.indirect_dma_start(
                        out=cs[:, c, :],
                        out_offset=None,
                        in_=cstab[:],
                        in_offset=bass.IndirectOffsetOnAxis(
                            ap=pidx[:, c:c + 1], axis=0),
                    )

                def emit_transposes(sc, roped):
                    for t in range(4):
                        tp = psmall.tile([128, 128], BF16, space="PSUM",
                                         tag="small")
                        nc.tensor.transpose(tp[:], roped[:, ts(t, 128)],
                                            ident[:])
                        dst = qt if t < 2 else kt
                        nc.scalar.copy(dst[:, t % 2, ts(sc, 128)], tp[:])

                pend_tp = None
                for sc in range(NSC):
                    ps = pbig.tile([128, QKV_COLS], F32, space="PSUM",
                                   tag="big")
                    for k in range(8):
                        nc.tensor.matmul(
                            ps[:, 0:512], lhsT=xa[:, k, ts(sc, 128)],
                            rhs=wt[:, k, 0:512],
                            start=(k == 0), stop=(k == 7),
                        )
                    for k in range(8):
                        nc.tensor.matmul(
                            ps[:, 512:768], lhsT=xa[:, k, ts(sc, 128)],
                            rhs=wt[:, k, 512:768],
                            start=(k == 0), stop=(k == 7),
                        )
                    if pend_tp is not None:
                        emit_transposes(*pend_tp)

                    # RoPE over the Q,K halves (cols 0:512), 8 blocks of 64
                    def qk_ap(off):
                        a = ps[:]
                        return bass.AP(a.tensor, a.offset + off,
                                       [a.ap[0], [DK, 2 * HPC], [2, DK // 2]])

                    def cs_ap(off):
                        a = cs[:, sc, :]
                        return bass.AP(a.tensor, a.offset + off,
                                       [a.ap[0], [0, 2 * HPC], [2, DK // 2]])

                    t1 = ropet.tile([128, 2 * HPC, DK // 2], F32, tag="t1")
                    t2 = ropet.tile([128, 2 * HPC, DK // 2], F32, tag="t2")
                    t3 = ropet.tile([128, 2 * HPC, DK // 2], F32, tag="t3")
                    t4 = ropet.tile([128, 2 * HPC, DK // 2], F32, tag="t4")
                    roped = ropet.tile([128, QK_COLS], BF16, tag="roped")

                    def roped_ap(off):
                        a = roped[:]
                        return bass.AP(a.tensor, a.offset + off,
                                       [a.ap[0], [DK, 2 * HPC], [2, DK // 2]])

                    nc.vector.tensor_tensor(t1[:], qk_ap(0), cs_ap(0), MUL)
                    nc.vector.tensor_tensor(t2[:], qk_ap(1), cs_ap(DK), MUL)
                    nc.vector.tensor_tensor(roped_ap(0), t1[:], t2[:], SUB)
                    nc.vector.tensor_tensor(t3[:], qk_ap(0), cs_ap(DK), MUL)
                    nc.vector.tensor_tensor(t4[:], qk_ap(1), cs_ap(0), MUL)
                    nc.vector.tensor_tensor(roped_ap(1), t3[:], t4[:], ADD)

                    # V columns (ones col + pad already set); ACT copy keeps
                    # the vector engine free for the RoPE chain
                    nc.scalar.copy(
                        vv4[:, sc, :, 0:DK],
                        ps[:, 512:768].rearrange("p (h e) -> p h e", h=HPC),
                    )

                    # transpose roped q/k after the NEXT chunk's matmuls so
                    # the PE never waits on the vector engine
                    pend_tp = (sc, roped)
                emit_transposes(*pend_tp)

                # wo load deferred here: overlaps attention, clear of the
                # startup critical path
                nc.scalar.dma_start(
                    out=wo[:, :, :],
                    in_=woT[:].rearrange("(k p) e -> p k e", p=128))

            # ------------- attention (head pairs) + split AllToAll ---------
            # a2ain[p] rows (u*512 + j*128 + d): q-slice j's rows for this
            # core's pair p, duplicated u=0/1 so block (4b + j) is correct
            # for either batch half b (SPMD-uniform staging).  After the
            # 8-core AllToAll, out block (4b + g) holds core (b, g)'s pair-p
            # rows for THIS core's q-slice == d-model chunk dp = 2g + p.
            a2ain = [dramp.tile([N_CORES * 128, QSLICE], BF16,
                                name=f"a2ain{p}") for p in range(2)]
            a2aout = [dramp.tile([N_CORES * 128, QSLICE], BF16,
                                 name=f"a2aout{p}") for p in range(2)]

            with tc.tile_pool(name="post", bufs=1) as postp:
                def emit_scores(p, j, eta, etb):
                    """Score matmuls + exp for chunk j, both heads of pair p
                    interleaved (row strips 0 / 64)."""
                    for qh in range(2):
                        q0 = max(128 * j, 1024 * qh)
                        q1 = 1024 * (qh + 1)
                        if q0 >= q1:
                            continue
                        sps = []
                        for i in range(2):
                            sp = pbig.tile([128, 1024], F32, space="PSUM",
                                           tag="big", name=f"sp{i}")
                            sps.append(sp)
                        if _NOILV:
                            for i, hf in enumerate((0, DK)):
                                for qq in (1024 * qh, 1024 * qh + 512):
                                    a, bnd = max(q0, qq), min(q1, qq + 512)
                                    if a >= bnd:
                                        continue
                                    nc.tensor.matmul(
                                        sps[i][:, ds(a - 1024 * qh, bnd - a)],
                                        lhsT=kt[ds(hf, DK), p, ts(j, 128)],
                                        rhs=qt[ds(hf, DK), p, ds(a, bnd - a)],
                                        start=True, stop=True,
                                    )
                        else:
                            for qq in (1024 * qh, 1024 * qh + 512):
                                a, bnd = max(q0, qq), min(q1, qq + 512)
                                if a >= bnd:
                                    continue
                                for i, hf in enumerate((0, DK)):
                                    nc.tensor.matmul(
                                        sps[i][:, ds(a - 1024 * qh, bnd - a)],
                                        lhsT=kt[ds(hf, DK), p, ts(j, 128)],
                                        rhs=qt[ds(hf, DK), p, ds(a, bnd - a)],
                                        start=True, stop=True,
                                    )
                        for i, et in enumerate((eta, etb)):
                            nc.scalar.activation(
                                et[:, ds(q0 - 128 * j, q1 - q0)],
                                sps[i][:, ds(q0 - 1024 * qh, q1 - q0)],
                                mybir.ActivationFunctionType.Exp,
                            )
                    # mask the diagonal block (q < k -> 0)
                    for et in (eta, etb):
                        nc.vector.tensor_tensor(
                            et[:, 0:128], et[:, 0:128], tri_t[:], MUL)

                def emit_av(h, qc, ets):
                    """Accumulate out^T for q-chunk qc over the whole causal
                    k-range in one PSUM tile."""
                    jmax = 4 * qc + 3
                    lw = VW if _NOPAD else 128
                    part = psmall.tile([128, 512], F32, space="PSUM",
                                       tag="small")
                    for j in range(jmax + 1):
                        a = max(512 * qc, 128 * j)
                        w = 512 * (qc + 1) - a
                        nc.tensor.matmul(
                            part[0:lw, ds(a - 512 * qc, w)],
                            lhsT=vv[:, j, ds(VW * h, lw)],
                            rhs=ets[j][:, ds(a - 128 * j, w)],
                            start=(j == 0), stop=(j == jmax),
                        )
                    nc.vector.tensor_copy(
                        outv[0:DK, h % 2, ts(qc, 512)], part[0:DK, :])
                    # pipelined per-chunk reciprocal of the sums row; the
                    # custom DVE op only works at partition 0 / offset 0
                    nc.vector.tensor_copy(sct[:], part[ds(DK, 1), :])
                    nc.vector.reciprocal_approx_fast(sct[:], sct[:])
                    nc.vector.tensor_copy(srw[:, h % 2, ts(qc, 512)], sct[:])

                def emit_norm_stage(p, h, qc):
                    """Normalize one q-chunk of head h into a scratch tile
                    and immediately stage it into the AllToAll input
                    (duplicated u=0/1), so the pair's collective can trigger
                    the moment its last AV chunk lands.  Scratch tiles (not
                    one big attT) keep the stage-DMA reads from serializing
                    against later chunks' normalize writes."""
                    hf = (h % 2) * DK
                    rb = psmall.tile([DK, 512], F32, space="PSUM",
                                     tag="small")
                    nc.tensor.matmul(rb[:], lhsT=ones1[:],
                                     rhs=srw[:, h % 2, ts(qc, 512)],
                                     start=True, stop=True)
                    sg = stgp.tile([DK, 512], BF16, tag="sg")
                    nc.vector.tensor_tensor(
                        sg[:], outv[0:DK, h % 2, ts(qc, 512)], rb[:], MUL)
                    stage = a2ain[p][:].rearrange(
                        "(u j d) q -> u d j q", u=2, j=GROUPS)
                    for u in range(2):
                        nc.sync.dma_start(
                            out=stage[u, ds(hf, DK), qc, :],
                            in_=sg[:],
                        )

                # gather-offset table [r, g] = 128*(4b + g) + r; single
                # small DMA on the gpsimd queue (idle through attention)
                nc.gpsimd.dma_start(out=ridx_sb[:, :], in_=ridx[:, :])

                with tc.tile_pool(name="etp", bufs=1) as etp:
                    for p in range(2):
                        ha, hb = 2 * p, 2 * p + 1
                        eta, etb = [], []
                        for j in range(NSC):
                            w = S - 128 * j
                            eta.append(etp.tile([128, w], BF16,
                                                name=f"eta{j}",
                                                tag=f"eta{j}"))
                            etb.append(etp.tile([128, w], BF16,
                                                name=f"etb{j}",
                                                tag=f"etb{j}"))
                        # norm+stage trails its AV by one unit so the PE
                        # never waits on the DVE reciprocal chain between
                        # consecutive AV accumulations (keeps PE warm, and
                        # the pair's collective triggers ~1 unit after the
                        # last AV instead of serializing 4 units)
                        avsched = {7: (ha, 0), 8: (hb, 0), 11: (ha, 1),
                                   12: (hb, 1), 13: (ha, 2), 14: (hb, 2)}
                        pend_ns = None
                        for j in range(NSC):
                            emit_scores(p, j, eta[j], etb[j])
                            if j in avsched:
                                h, qc = avsched[j]
                                emit_av(h, qc, eta if h == ha else etb)
                                if pend_ns is not None:
                                    emit_norm_stage(p, *pend_ns)
                                pend_ns = (h, qc)
                        emit_av(ha, 3, eta)
                        emit_norm_stage(p, *pend_ns)
                        emit_av(hb, 3, etb)
                        emit_norm_stage(p, ha, 3)
                        emit_norm_stage(p, hb, 3)
                        if _DBG and p == 0:
                            nc.sync.dma_start(
                                out=dbgraw[:].rearrange(
                                    "p (a b) -> p a b", a=2),
                                in_=outv[:, :, :])
                            nc.sync.dma_start(out=dbget[:], in_=eta[0][:])
                        nc.gpsimd.collective_compute(
                            "AllToAll",
                            mybir.AluOpType.bypass,
                            ins=[a2ain[p][:]],
                            outs=[a2aout[p][:]],
                            replica_groups=[list(range(N_CORES))],
                        )

                # ------------- output projection (q-slice) ------------------
                # Release the attention PSUM pools and claim all 8 banks as
                # one persistent accumulator: each ec-chunk accumulates its
                # full 8-chunk contraction in its own bank, dp-major so the
                # even-dp half (pair 0, available early) runs during the
                # pair-1 collective wait, and each odd-dp gather overlaps the
                # previous dp's matmuls.  The bank reuse also hard-orders
                # this work after attention, keeping the scheduler from
                # stealing PE time from the collective's critical path.
                pstack.close()
                with tc.tile_pool(name="ptail", bufs=8, space="PSUM") as pt:
                    fps = [pt.tile([128, QSLICE], F32, space="PSUM",
                                   tag="fp", name=f"fp{ec}")
                           for ec in range(8)]
                    dporder = (0, 2, 4, 6, 1, 3, 5, 7)
                    for i, dp in enumerate(dporder):
                        nc.gpsimd.indirect_dma_start(
                            out=rwo[:, dp, :],
                            out_offset=None,
                            in_=a2aout[dp % 2][:],
                            in_offset=bass.IndirectOffsetOnAxis(
                                ap=ridx_sb[:, dp // 2:dp // 2 + 1], axis=0),
                        )
                        for ec in range(8):
                            nc.tensor.matmul(
                                fps[ec][:], lhsT=wo[:, dp, ts(ec, 128)],
                                rhs=rwo[:, dp, :],
                                start=(i == 0), stop=(i == 7),
                            )
                    for ec in range(8):
                        fin_sb = xtp.tile([128, QSLICE], BF16, tag="fin")
                        # alternate engines/queues so the 8 stores pipeline
                        if ec % 2 == 0:
                            nc.vector.tensor_copy(fin_sb[:], fps[ec][:])
                            nc.sync.dma_start(out=finT[ts(ec, 128), :],
                                              in_=fin_sb[:])
                        else:
                            nc.scalar.copy(fin_sb[:], fps[ec][:])
                            nc.scalar.dma_start(out=finT[ts(ec, 128), :],
                                                in_=fin_sb[:])

    nc.compile()
    return nc


def _host_prep(x, token_positions, W_qkv, W_o):
    bf16 = ml_dtypes.bfloat16
    xT = np.ascontiguousarray(np.transpose(x, (0, 2, 1))).astype(bf16)  # [B,D,S]

    # per-group W_qkv^T slices (Q rows pre-scaled by 1/sqrt(dk))
    wq = W_qkv[0 * D:1 * D] * np.float32(1.0 / np.sqrt(DK))
    wk = W_qkv[1 * D:2 * D]
    wv = W_qkv[2 * D:3 * D]
    wslices = []
    for g in range(GROUPS):
        rows = slice(g * HPC * DK, (g + 1) * HPC * DK)
        wsl = np.concatenate([wq[rows], wk[rows], wv[rows]], axis=0)  # [768, D]
        wslices.append(np.ascontiguousarray(wsl.T).astype(bf16))      # [D, 768]

    woT = np.ascontiguousarray(W_o.T).astype(bf16)                    # [D, D]

    idx = np.arange(DK // 2, dtype=np.float64)
    freqs = 1.0 / (THETA ** (2.0 * idx / DK))
    ang = np.arange(MAXPOS, dtype=np.float64)[:, None] * freqs[None, :]
    cstab = np.zeros((MAXPOS, 2 * DK), dtype=np.float32)
    cstab[:, 0:DK:2] = np.cos(ang)
    cstab[:, 1:DK:2] = np.cos(ang)
    cstab[:, DK::2] = np.sin(ang)
    cstab[:, DK + 1::2] = np.sin(ang)

    tri = (np.arange(128)[None, :] >= np.arange(128)[:, None]).astype(bf16)

    # pos pre-rearranged to [partition, chunk]: pos_r[p, c] = pos[128c + p]
    # (contiguous 64B per partition -> cheap DMA)
    posi = np.ascontiguousarray(
        np.asarray(token_positions).astype(np.int32).reshape(B, NSC, 128)
        .transpose(0, 2, 1))

    # ridx[r, g'] = row 128*(4b + g') + r of a2aout[p]: source core (b, g')'s
    # pair-p rows for this core's q-slice = d-model chunk dp = 2g' + p.
    rr = np.arange(128)
    in_maps = []
    for c in range(N_CORES):
        b, g = c // GROUPS, c % GROUPS
        ridx = 128 * (4 * b + np.arange(GROUPS))[None, :] + rr[:, None]
        in_maps.append({
            "xT": np.asarray(xT[b]),
            "wqkvT": wslices[g],
            "woT": woT,
            "cstab": cstab,
            "pos": np.ascontiguousarray(posi[b]),
            "tri": tri,
            "ridx": np.ascontiguousarray(ridx.astype(np.int32)),
        })
    return in_maps


def _assemble(results):
    out = np.empty((B, S, D), dtype=np.float32)
    for b in range(B):
        fullT = np.concatenate(
            [results[b * GROUPS + g]["finT"].astype(np.float32)
             for g in range(GROUPS)], axis=1)
        out[b] = fullT.T
    return out


_NC_CACHE = {}


def run(inputs, trace=False, **kw):
    if "nc" not in _NC_CACHE:
        _NC_CACHE["nc"] = _build()
    nc = _NC_CACHE["nc"]
    in_maps = _host_prep(**inputs)
    res = run_bass_kernel_spmd(
        nc, in_maps, core_ids=list(range(N_CORES)), trace=trace, **kw)
    return _assemble(res.results), res


def kernel(**inputs):
    out, _ = run(inputs, trace=False)
    return out

